# revision 1
# baseline (speedup 1.0000x reference)
"""Trainium2 Bass kernel for nn_DependencyEncoder (shift-reduce tree-LSTM).

Structure exploited (validated vs reference): transitions strictly alternate
shift/reduce, so stack[0] is frozen at token0, stack[1] holds one running
composed value v, and the module collapses to K=63 iterations of
  a (shift  t=2k  ): tracking LSTM on x=[tok_k, v, tok_0]
  b (reduce t=2k+1): tracking LSTM on x=[tok_{k+1}, tok_k, v]
  comp             : gates = Wl@(ml*tok_k) + Wr@(mr*v) + W_c@th_b,
                     c' = si*tanh(u) + (sfh+sfc)*c_head,  v <- (h', c')
Output = v_h after the last pair.

Kernel layout (v2): everything GATE-MAJOR / FEATURE-MAJOR, all matmuls fp16.
  - matmul outputs put gate rows on PSUM partitions and batch on the free
    dim, so each matmul streams only ~24 moving columns (bf16 runs full
    rate at any stream size; fp32r would be 1/4 rate below 256).  No PE
    transposes anywhere: the elementwise tails produce th and v_h directly
    in the feature-major layout the next matmuls consume.
  - per core the 48-row batch is split into 2 chains of 24 that run the
    serial recurrence staggered, so one chain's activation-engine round
    trips hide under the other chain's work.
  - left-masked heads (ml*tok_h), left c-head parts (ml*tok_c) and the
    right-mask broadcast are precomputed on the host (pure input prep);
    their matmuls are chain-independent and are emitted ahead as PE filler
    so the tensor engine never idles (keeps the 2.4 GHz p-state).
  - tanh via 2*sigmoid(2x)-1 with u/g weight rows pre-scaled by 2 so the
    Activation engine only ever runs Sigmoid (one act table load).

Sharding: pure batch data parallelism, 384 rows -> 8 cores x 48 rows.
"""
import numpy as np

import concourse.bacc as bacc
import concourse.mybir as mybir
import concourse.tile as tile
from concourse.alu_op_type import AluOpType as alu
from concourse.bass_utils import run_bass_kernel_spmd

AF = mybir.ActivationFunctionType
f32 = mybir.dt.float32
bf16 = mybir.dt.float16
BFNP = mybir.dt.np(bf16)   # np.float16

B_FULL, L, H, TD = 384, 64, 300, 64
NCORES = 8
B = B_FULL // NCORES            # 48 rows per core
NCH = 2                         # serial chains per core
CB = B // NCH                   # 24 rows per chain
K = int(__import__("os").environ.get("KERNEL_PAIRS", L - 1))   # 63 pairs
CH = [(0, 128), (128, 128), (256, 44)]   # feature chunks (offset, size)
NT = 15                         # comp gate tiles: 5 gates x 3 feature chunks
PD = 1                          # pcc psum prefetch depth (pairs ahead)


# --------------------------------------------------------------------------
# host-side input preparation
# --------------------------------------------------------------------------
def _track_w(W_ih, W_hh):
    """torch gate rows (i,f,g,o) -> stationary tiles [128, 10, 2, 128] bf16.
    tile0 cols = [i | f], tile1 cols = [2*g | o]; chunk q=seg*3+c over the
    900-dim x, q=9 is the 64-row U_hh chunk (zero-padded to 128)."""
    Wr = np.concatenate([W_ih[0:64], W_ih[64:128],
                         2.0 * W_ih[128:192], W_ih[192:256]], axis=0)
    Uh = np.concatenate([W_hh[0:64], W_hh[64:128],
                         2.0 * W_hh[128:192], W_hh[192:256]], axis=0)
    out = np.zeros((128, 10, 2, 128), np.float32)
    for s in range(3):
        for c, (off, sz) in enumerate(CH):
            blk = Wr[:, s * H + off: s * H + off + sz]      # [256, sz]
            out[:sz, s * 3 + c, 0, :] = blk[0:128].T
            out[:sz, s * 3 + c, 1, :] = blk[128:256].T
    out[:TD, 9, 0, :] = Uh[0:128].T
    out[:TD, 9, 1, :] = Uh[128:256].T
    return out.astype(BFNP)


def _comp_w(W_c, Uh_w, Ul_w, Ur_w):
    """-> stationary tiles [128, 7, 15, 128] bf16.
    Gate rows reordered (i,o,fh,fc,u) -> (fh,fc,i,2u,o); out tile t=g*3+c,
    contraction chunk q: 0-2 = Wl=Uh+Ul over head, 3-5 = Wr=Uh+Ur, 6 = W_c
    (64 rows).  All tiles zero-padded to 128 cols/rows."""
    def reorder(W):
        g = W.reshape(5, H, -1)
        return np.concatenate([g[2], g[3], g[0], 2.0 * g[4], g[1]], axis=0)
    Wl = reorder(Uh_w + Ul_w)        # [1500, 300]
    Wr = reorder(Uh_w + Ur_w)
    Wc = reorder(W_c)                # [1500, 64]
    out = np.zeros((128, 7, NT, 128), np.float32)
    for g in range(5):
        for c, (off, sz) in enumerate(CH):
            t = g * 3 + c
            rows = slice(g * H + off, g * H + off + sz)     # gate rows of t
            for q, (qo, qs) in enumerate(CH):               # head chunks
                out[:qs, q, t, :sz] = Wl[rows, qo:qo + qs].T
                out[:qs, 3 + q, t, :sz] = Wr[rows, qo:qo + qs].T
            out[:TD, 6, t, :sz] = Wc[rows, :].T
    return out.astype(BFNP)


def _prep_core(inputs, s):
    """Per-core input map (batch slice s)."""
    seq = np.asarray(inputs["sequence"], np.float32)[s]      # [B, L, 600]
    tr = np.asarray(inputs["transitions"])[s]
    th0 = np.asarray(inputs["th0"], np.float32)[s]
    tc0 = np.asarray(inputs["tc0"], np.float32)[s]

    tok_h = seq[:, :, :H]                                    # [B, L, 300]
    tok_c = seq[:, :, H:]
    is_left = (tr[:, 1::2].T == 2).astype(np.float32)[:K]    # [K, B]
    ml = is_left[None, :, None, :]                           # [1, K, 1, B]
    mr = 1.0 - ml

    tokh = np.zeros((128, L, 3, B), np.float32)
    tokc = np.zeros((128, L, 3, B), np.float32)
    for c, (off, sz) in enumerate(CH):
        tokh[:sz, :, c] = tok_h[:, :, off:off + sz].transpose(2, 1, 0)
        tokc[:sz, :, c] = tok_c[:, :, off:off + sz].transpose(2, 1, 0)
    hl = tokh[:, :K] * ml                                    # [128, K, 3, B]
    chl = tokc[:, :K] * ml
    chl[:, 0] = tokc[:, 0]          # k=0: right head (v_c) is token0 too
    mr3 = np.broadcast_to(mr, (128, K, 3, B))

    return dict(
        tokh=tokh.astype(BFNP), hl=hl.astype(BFNP),
        chl=np.ascontiguousarray(chl, np.float32),
        mr3=np.ascontiguousarray(mr3.astype(BFNP)),
        th0=np.ascontiguousarray(th0.T.astype(BFNP)),        # [64, B]
        tc0=np.ascontiguousarray(tc0.T),                     # [64, B] f32
    )


# --------------------------------------------------------------------------
# device program
# --------------------------------------------------------------------------
_CACHED_NC = None
CUR = ['init']            # label cell for analysis tooling


def _build_nc():
    nc = bacc.Bacc("TRN2", target_bir_lowering=False)
    tokh_d = nc.dram_tensor("tokh", [128, L, 3, B], bf16, kind="ExternalInput")
    hl_d = nc.dram_tensor("hl", [128, K, 3, B], bf16, kind="ExternalInput")
    chl_d = nc.dram_tensor("chl", [128, K, 3, B], f32, kind="ExternalInput")
    mr3_d = nc.dram_tensor("mr3", [128, K, 3, B], bf16, kind="ExternalInput")
    wtrk_d = nc.dram_tensor("wtrk", [128, 10, 2, 128], bf16,
                            kind="ExternalInput")
    wcmp_d = nc.dram_tensor("wcmp", [128, 7, NT, 128], bf16,
                            kind="ExternalInput")
    th0_d = nc.dram_tensor("th0", [TD, B], bf16, kind="ExternalInput")
    tc0_d = nc.dram_tensor("tc0", [TD, B], f32, kind="ExternalInput")
    outh_d = nc.dram_tensor("outh", [128, 3, B], f32, kind="ExternalOutput")
    DBG = __import__("os").environ.get("DEBUG_TAPS", "0") == "1"
    if DBG:
        dbg_psa = nc.dram_tensor("dbg_psa", [128, 4, 2, CB], f32,
                                 kind="ExternalOutput")
        dbg_sa = nc.dram_tensor("dbg_sa", [128, 2, 2, CB], f32,
                                kind="ExternalOutput")
        dbg_th = nc.dram_tensor("dbg_th", [TD, 2, 2, CB], f32,
                                kind="ExternalOutput")
        dbg_tc = nc.dram_tensor("dbg_tc", [TD, 2, 2, CB], f32,
                                kind="ExternalOutput")
        dbg_pcc = nc.dram_tensor("dbg_pcc", [128, NT, CB], f32,
                                 kind="ExternalOutput")
        dbg_hr = nc.dram_tensor("dbg_hr", [128, 3, CB], f32,
                                kind="ExternalOutput")
        dbg_wtrk = nc.dram_tensor("dbg_wtrk", [128, 10, 2, 128], bf16,
                                  kind="ExternalOutput")
        dbg_tokh = nc.dram_tensor("dbg_tokh", [128, 4, 3, B], bf16,
                                  kind="ExternalOutput")
        dbg_th0 = nc.dram_tensor("dbg_th0", [TD, B], bf16,
                                 kind="ExternalOutput")

    with tile.TileContext(nc) as tc_:
        with (
            tc_.tile_pool(name="sg", bufs=1) as sg,
            tc_.tile_pool(name="st", bufs=3) as st,
            tc_.tile_pool(name="psc", bufs=2, space="PSUM") as psc,
            tc_.tile_pool(name="pst", bufs=2, space="PSUM") as pst,
        ):
            # ---------------- resident inputs ----------------
            tokh = sg.tile([128, L, 3, B], bf16)
            hl = sg.tile([128, K, 3, B], bf16)
            chl = sg.tile([128, K, 3, B], f32)
            mr3 = sg.tile([128, K, 3, B], bf16)
            wtrk = sg.tile([128, 10, 2, 128], bf16)
            wcmp = sg.tile([128, 7, NT, 128], bf16)
            th0 = sg.tile([TD, B], bf16)
            tc0 = sg.tile([128, B], f32)          # value at partitions 64:128

            nc.sync.dma_start(wtrk[:], wtrk_d[:])
            nc.sync.dma_start(wcmp[:], wcmp_d[:])
            nc.sync.dma_start(th0[:], th0_d[:])
            nc.sync.dma_start(tc0[TD:128, :], tc0_d[:])
            NG = 8
            for g in range(NG):
                lo, hi = g * (L // NG), (g + 1) * (L // NG)
                nc.sync.dma_start(tokh[:, lo:hi], tokh_d[:, lo:hi])
                klo, khi = min(lo, K), min(hi, K)
                if khi > klo:
                    nc.sync.dma_start(hl[:, klo:khi], hl_d[:, klo:khi])
                    nc.sync.dma_start(chl[:, klo:khi], chl_d[:, klo:khi])
                    nc.sync.dma_start(mr3[:, klo:khi], mr3_d[:, klo:khi])

            mm = nc.tensor.matmul

            chains = []
            for ci in range(NCH):
                sl = slice(ci * CB, (ci + 1) * CB)
                chains.append(dict(
                    ci=ci, sl=sl,
                    th=th0[:, sl],                  # bf16 [64, 24] moving
                    tc=tc0[:, sl],                  # f32, value at parts 64:
                    v=[tokh[0:sz, 0, c, sl] for c, (_, sz) in enumerate(CH)],
                    vc=None, vh3=None, hr=None, chd=None,
                    psa=None, psb=None, pcc={}, tha=None,
                ))

            # ---------------- emission helpers ----------------
            trk_tiles = {}

            def track_toks(ch, k, step):
                """Token mms opening the (k, step) track psum groups; the
                t0/t1 gate tiles sit in different PSUM banks (512-column
                stride) so the per-bank group-window rule applies per slot
                sequence, verified on hardware."""
                ci = ch["ci"]
                CUR[0] = f'ttok{ci}.{k}'
                if k not in trk_tiles:
                    tnew = pst.tile([128, 2, 512], f32, tag="trk")
                    trk_tiles[k] = tnew
                slot = step * 2 + ci
                ps = trk_tiles[k][:, :, slot * CB:(slot + 1) * CB]
                sl = ch["sl"]
                segs = [(0, k), (6, 0)] if step == 0 else [(0, k + 1), (3, k)]
                first = True
                for q0, kk in segs:
                    for c, (_, sz) in enumerate(CH):
                        for t in range(2):
                            mm(ps[:, t, :], wtrk[0:sz, q0 + c, t, :],
                               tokh[0:sz, kk, c, sl], start=first, stop=False)
                        first = False
                return ps

            def track_dyn(ch, ps, step):
                """v- and th-dependent mms closing both tile groups."""
                CUR[0] = f'tdyn{ch["ci"]}'
                qv = 3 if step == 0 else 6
                if step == 0:
                    for t in range(2):
                        mm(ps[:, t, :], wtrk[0:TD, 9, t, :], ch["th"],
                           start=False, stop=False)
                    for c, (_, sz) in enumerate(CH):
                        for t in range(2):
                            mm(ps[:, t, :], wtrk[0:sz, qv + c, t, :],
                               ch["v"][c], start=False, stop=(c == 2))
                else:
                    for c, (_, sz) in enumerate(CH):
                        for t in range(2):
                            mm(ps[:, t, :], wtrk[0:sz, qv + c, t, :],
                               ch["v"][c], start=False, stop=False)
                    for t in range(2):
                        mm(ps[:, t, :], wtrk[0:TD, 9, t, :], ch["th"],
                           start=False, stop=True)

            def track_group(ch, k, step):
                ps = track_toks(ch, k, step)
                track_dyn(ch, ps, step)
                return ps

            def track_step(ch, ps, step):
                """Sigmoid + LSTM cell tail. sa: [:,0]=[i|f], [:,1]=[2g|o].
                tc state lives at partitions 64:128; th output at 0:64."""
                ci = ch["ci"]
                CUR[0] = f'ttail{step}{ci}'
                sa = st.tile([128, 2, CB], f32, tag=f"sa{step}{ci}")
                nc.scalar.activation(sa[:], ps[:], AF.Sigmoid)
                si = sa[0:TD, 0, :]
                sf = sa[TD:128, 0, :]
                sg2 = sa[0:TD, 1, :]
                so = sa[TD:128, 1, :]
                m = st.tile([TD, CB], f32, tag=f"m{ci}")
                nc.vector.tensor_tensor(m[:], si, sg2, alu.mult)
                q = st.tile([128, CB], f32, tag=f"q{ci}")
                nc.vector.scalar_tensor_tensor(q[TD:128, :], m[:], 2.0, si,
                                               alu.mult, alu.subtract)
                r = st.tile([128, CB], f32, tag=f"r{ci}")
                nc.gpsimd.tensor_tensor(r[TD:128, :], sf, ch["tc"][TD:128, :],
                                        alu.mult)
                tcn = st.tile([128, CB], f32, tag=f"tc{ci}")
                nc.vector.tensor_tensor(tcn[TD:128, :], q[TD:128, :],
                                        r[TD:128, :], alu.add)
                ttc = st.tile([128, CB], f32, tag=f"stc{ci}")
                nc.scalar.activation(ttc[TD:128, :], tcn[TD:128, :], AF.Tanh)
                thn = st.tile([TD, CB], bf16, tag=f"th{step}{ci}")
                nc.vector.tensor_tensor(thn[:], so, ttc[TD:128, :], alu.mult)
                if DBG and ci == 0 and ch.get("dbgk") == 0:
                    psf = st.tile([128, 2, CB], f32, tag=f"dpsf{step}")
                    nc.vector.tensor_copy(psf[:], ps[:])
                    nc.sync.dma_start(dbg_psa[:, step * 2], psf[:])
                    thf = st.tile([TD, CB], f32, tag=f"dthf{step}")
                    nc.vector.tensor_tensor(thf[:], so, ttc[TD:128, :],
                                            alu.mult)
                    nc.sync.dma_start(dbg_sa[:, step], sa[:])
                    nc.sync.dma_start(dbg_th[:, step, 0], thf[:])
                    nc.sync.dma_start(dbg_tc[:, step, 0], tcn[TD:128, :])
                ch["tc"] = tcn
                ch["th"] = thn[:]

            def comp_mms(ch, k):
                """pcc(k): per gate tile ONE contiguous accumulation group
                [W_c@th_b, hl x3, hr x3].  PSUM group state is bank-scoped,
                so groups sharing the bank must be strictly sequential; the
                price is that the whole fill is gated on th_b."""
                ci = ch["ci"]
                CUR[0] = f'cmm{ci}.{k}'
                sl = ch["sl"]
                pcc = psc.tile([128, NT, CB], f32, tag=f"pcc{ci}")
                ch["pcc"][k] = pcc
                hr = ch["hr"]
                for t in range(NT):
                    mm(pcc[:, t, :], wcmp[0:TD, 6, t, :], ch["th"],
                       start=True, stop=False)
                    for c, (_, sz) in enumerate(CH):
                        mm(pcc[:, t, :], wcmp[0:sz, c, t, :],
                           hl[0:sz, k, c, sl], start=False, stop=False)
                    for c, (_, sz) in enumerate(CH):
                        mm(pcc[:, t, :], wcmp[0:sz, 3 + c, t, :],
                           hr[0:sz, c, :], start=False, stop=(c == 2))
                return pcc

            def comp_op_hr(ch, k):
                """hr = mr3[k] * v   (f16, one op over all chunks)."""
                ci = ch["ci"]
                CUR[0] = f'hrop{ci}.{k}'
                sl = ch["sl"]
                hr = st.tile([128, 3, CB], bf16, tag=f"hr{ci}")
                vsrc = tokh[:, 0, :, sl] if k == 0 else ch["vh3"][:]
                nc.vector.tensor_tensor(hr[:], vsrc, mr3[:, k, :, sl],
                                        alu.mult)
                ch["hr"] = hr

            def comp_op_ch(ch, k):
                """c_head = chl[k] + mr3[k]*vc   (f32, [128,3,24])."""
                ci = ch["ci"]
                CUR[0] = f'chop{ci}.{k}'
                sl = ch["sl"]
                if k == 0:
                    ch["chd"] = chl[:, 0, :, sl]
                    return
                w = st.tile([128, 3, CB], f32, tag=f"w{ci}")
                nc.gpsimd.tensor_tensor(w[:], ch["vc"][:], mr3[:, k, :, sl],
                                        alu.mult)
                chd = st.tile([128, 3, CB], f32, tag=f"chd{ci}")
                nc.vector.tensor_tensor(chd[:], w[:], chl[:, k, :, sl],
                                        alu.add)
                ch["chd"] = chd[:]

            def comp_tail(ch, k):
                """Composition cell tail; produces vh3 (bf16) + vc (f32)."""
                ci = ch["ci"]
                CUR[0] = f'ctail{ci}.{k}'
                pcc = ch["pcc"].pop(k)
                if DBG and ci == 0 and k == 0:
                    pccf = st.tile([128, NT, CB], f32, tag="dpccf")
                    nc.vector.tensor_copy(pccf[:], pcc[:])
                    nc.sync.dma_start(dbg_pcc[:], pccf[:])
                    hrf = st.tile([128, 3, CB], f32, tag="dhrf")
                    nc.vector.tensor_copy(hrf[:], ch["hr"][:])
                    nc.sync.dma_start(dbg_hr[:], hrf[:])
                sc = st.tile([128, NT, CB], f32, tag=f"sc{ci}")
                # tiles: 0:3 fh, 3:6 fc, 6:9 i, 9:12 2u, 12:15 o
                nc.scalar.activation(sc[:, 0:12, :], pcc[:, 0:12, :],
                                     AF.Sigmoid)
                nc.scalar.activation(sc[:, 12:15, :], pcc[:, 12:15, :],
                                     AF.Sigmoid)
                A = st.tile([128, 3, CB], f32, tag=f"A{ci}")
                nc.gpsimd.tensor_tensor(A[:], sc[:, 0:3, :], sc[:, 3:6, :],
                                        alu.add)
                t3 = st.tile([128, 3, CB], f32, tag=f"t3{ci}")
                nc.gpsimd.tensor_tensor(t3[:], A[:], ch["chd"], alu.mult)
                m2 = st.tile([128, 3, CB], f32, tag=f"m2{ci}")
                nc.vector.tensor_tensor(m2[:], sc[:, 6:9, :], sc[:, 9:12, :],
                                        alu.mult)
                q2 = st.tile([128, 3, CB], f32, tag=f"q2{ci}")
                nc.vector.scalar_tensor_tensor(q2[:], m2[:], 2.0,
                                               sc[:, 6:9, :],
                                               alu.mult, alu.subtract)
                cj = st.tile([128, 3, CB], f32, tag=f"cj{ci}")
                nc.vector.tensor_tensor(cj[:], q2[:], t3[:], alu.add)
                tcc = st.tile([128, 3, CB], f32, tag=f"scc{ci}")
                nc.scalar.activation(tcc[:], cj[:], AF.Tanh)
                vh3 = st.tile([128, 3, CB], bf16, tag=f"vh3{ci}")
                nc.vector.tensor_tensor(vh3[:], sc[:, 12:15, :], tcc[:],
                                        alu.mult)
                ch["vh3"] = vh3
                ch["vc"] = cj
                ch["v"] = [vh3[0:sz, c, :] for c, (_, sz) in enumerate(CH)]
                if k == K - 1:
                    vhf = st.tile([128, 3, CB], f32, tag=f"vhf{ci}")
                    nc.vector.tensor_tensor(vhf[:], sc[:, 12:15, :], tcc[:],
                                            alu.mult)
                    nc.sync.dma_start(outh_d[:, :, ch["sl"]], vhf[:])

            # ---------------- main schedule ----------------
            A, Bc = chains
            if DBG:
                nc.sync.dma_start(dbg_wtrk[:], wtrk[:])
                nc.sync.dma_start(dbg_tokh[:], tokh[:, 0:4])
                nc.sync.dma_start(dbg_th0[:], th0[:])

            # warmup
            for ch in chains:
                comp_op_hr(ch, 0)
                comp_op_ch(ch, 0)
            A["psa"] = track_group(A, 0, 0)
            Bc["psa"] = track_group(Bc, 0, 0)

            for k in range(K):
                for ch in chains:
                    ch["dbgk"] = k
                track_step(A, A["psa"], 0)          # sigma-a(A) -> th_a(A)
                if k + 1 < K:
                    A["npsa"] = track_toks(A, k + 1, 0)   # window in tile k+1
                track_step(Bc, Bc["psa"], 0)
                A["psb"] = track_group(A, k, 1)     # closes at th_a(A)
                Bc["psb"] = track_group(Bc, k, 1)
                track_step(A, A["psb"], 1)          # sigma-b(A) -> th_b(A)
                comp_mms(A, k)
                track_step(Bc, Bc["psb"], 1)
                comp_mms(Bc, k)
                comp_tail(A, k)                     # sigma-c(A) -> v(A,k)
                if k + 1 < K:
                    comp_op_hr(A, k + 1)
                    comp_op_ch(A, k + 1)
                    A["psa"] = A.pop("npsa")
                    track_dyn(A, A["psa"], 0)       # closes A-a(k+1) window
                comp_tail(Bc, k)
                if k + 1 < K:
                    comp_op_hr(Bc, k + 1)
                    comp_op_ch(Bc, k + 1)
                    Bc["psa"] = track_group(Bc, k + 1, 0)  # contiguous

    nc.compile()
    return nc


def _get_nc():
    global _CACHED_NC
    if _CACHED_NC is None:
        _CACHED_NC = _build_nc()
    return _CACHED_NC


# --------------------------------------------------------------------------
# host wrapper
# --------------------------------------------------------------------------
def make_in_maps(inputs):
    wtrk = _track_w(np.asarray(inputs["W_ih"], np.float32),
                    np.asarray(inputs["W_hh"], np.float32))
    wcmp = _comp_w(np.asarray(inputs["W_c"], np.float32),
                   np.asarray(inputs["Uh_w"], np.float32),
                   np.asarray(inputs["Ul_w"], np.float32),
                   np.asarray(inputs["Ur_w"], np.float32))
    in_maps = []
    for i in range(NCORES):
        s = slice(i * B, (i + 1) * B)
        d = _prep_core(inputs, s)
        d.update(wtrk=wtrk, wcmp=wcmp)
        in_maps.append(d)
    return in_maps


def assemble_out(res_list):
    outs = []
    for r in res_list:
        arr = r["outh"]                      # [128, 3, B]
        o = np.empty((B, H), np.float32)
        for c, (off, sz) in enumerate(CH):
            o[:, off:off + sz] = arr[0:sz, c, :].T
        outs.append(o)
    return np.concatenate(outs, axis=0)


def kernel(**inputs) -> np.ndarray:
    nc = _get_nc()
    in_maps = make_in_maps(inputs)
    res = run_bass_kernel_spmd(nc, in_maps, core_ids=list(range(NCORES)))
    return assemble_out(res.results)



# revision 5
# speedup vs baseline: 1.3661x; 1.3661x over previous
"""Trainium2 Bass kernel for nn_DependencyEncoder (shift-reduce tree-LSTM).

Structure exploited (validated vs reference): transitions strictly alternate
shift/reduce, so stack[0] is frozen at token0, stack[1] holds one running
composed value v, and the module collapses to K=63 iterations of
  a (shift  t=2k  ): tracking LSTM on x=[tok_k, v, tok_0]
  b (reduce t=2k+1): tracking LSTM on x=[tok_{k+1}, tok_k, v]
  comp             : gates = Wl@(ml*tok_k) + Wr@(mr*v) + W_c@th_b,
                     c' = si*tanh(u) + (sfh+sfc)*c_head,  v <- (h', c')
Output = v_h after the last pair.

v3: the wall time is 63 x (serial loop latency), so the schedule minimizes
the recurrence critical path:
  - every PSUM accumulation is split into SHORT sequential windows per
    region ordered by data arrival (tok -> th -> v for tracking;
    hl -> hr -> W_c@th for composition), so a late term puts only its own
    1-3 matmuls + PE pipeline on the critical path instead of gating the
    whole fill (the old kernel put W_c@th FIRST, gating 105 mms on th_b).
  - each (chain, step) tracking accumulator and each chain's comp
    accumulator gets its own PSUM bank (2KB/partition), so the two batch
    chains' group windows never serialize against each other.
  - one merged 15-tile sigmoid for the composition gates; all sigmoid /
    elementwise intermediates in bf16 (DVE 2x/4x modes), tc/c_j kept f32.
  - tanh via 2*sigmoid(2x)-1 with u/g weight rows pre-scaled by 2 so the
    Activation engine only ever runs Sigmoid+Tanh (tanh used directly for
    tc / c_j, table stays loaded).

Sharding: pure batch data parallelism, 384 rows -> 8 cores x 48 rows,
2 independent chains of 24 rows per core.
"""
import numpy as np

import concourse.bacc as bacc
import concourse.mybir as mybir
import concourse.tile as tile
from concourse.alu_op_type import AluOpType as alu
from concourse.bass_utils import run_bass_kernel_spmd

AF = mybir.ActivationFunctionType
f32 = mybir.dt.float32
bf16 = mybir.dt.float16
BFNP = mybir.dt.np(bf16)   # np.float16

B_FULL, L, H, TD = 384, 64, 300, 64
NCORES = 8
B = B_FULL // NCORES            # 48 rows per core
NCH = 2                         # serial chains per core
CB = B // NCH                   # 24 rows per chain
K = int(__import__("os").environ.get("KERNEL_PAIRS", L - 1))   # 63 pairs
CH = [(0, 128), (128, 128), (256, 44)]   # feature chunks (offset, size)
NT = 15                         # comp gate tiles: 5 gates x 3 feature chunks


# --------------------------------------------------------------------------
# host-side input preparation
# --------------------------------------------------------------------------
def _track_w(W_ih, W_hh):
    """torch gate rows (i,f,g,o) -> stationary tiles [128, 10, 2, 128] bf16.
    tile0 cols = [i | f], tile1 cols = [2*g | o]; chunk q=seg*3+c over the
    900-dim x, q=9 is the 64-row U_hh chunk (zero-padded to 128)."""
    Wr = np.concatenate([W_ih[0:64], W_ih[64:128],
                         2.0 * W_ih[128:192], W_ih[192:256]], axis=0)
    Uh = np.concatenate([W_hh[0:64], W_hh[64:128],
                         2.0 * W_hh[128:192], W_hh[192:256]], axis=0)
    out = np.zeros((128, 10, 2, 128), np.float32)
    for s in range(3):
        for c, (off, sz) in enumerate(CH):
            blk = Wr[:, s * H + off: s * H + off + sz]      # [256, sz]
            out[:sz, s * 3 + c, 0, :] = blk[0:128].T
            out[:sz, s * 3 + c, 1, :] = blk[128:256].T
    out[:TD, 9, 0, :] = Uh[0:128].T
    out[:TD, 9, 1, :] = Uh[128:256].T
    return out.astype(BFNP)


def _comp_w(W_c, Uh_w, Ul_w, Ur_w):
    """-> stationary tiles [128, 7, 15, 128] bf16.
    Gate rows reordered (i,o,fh,fc,u) -> (fh,fc,i,2u,o); out tile t=g*3+c,
    contraction chunk q: 0-2 = Wl=Uh+Ul over head, 3-5 = Wr=Uh+Ur, 6 = W_c
    (64 rows).  All tiles zero-padded to 128 cols/rows."""
    def reorder(W):
        g = W.reshape(5, H, -1)
        return np.concatenate([g[2], g[3], g[0], 2.0 * g[4], g[1]], axis=0)
    Wl = reorder(Uh_w + Ul_w)        # [1500, 300]
    Wr = reorder(Uh_w + Ur_w)
    Wc = reorder(W_c)                # [1500, 64]
    out = np.zeros((128, 7, NT, 128), np.float32)
    for g in range(5):
        for c, (off, sz) in enumerate(CH):
            t = g * 3 + c
            rows = slice(g * H + off, g * H + off + sz)     # gate rows of t
            for q, (qo, qs) in enumerate(CH):               # head chunks
                out[:qs, q, t, :sz] = Wl[rows, qo:qo + qs].T
                out[:qs, 3 + q, t, :sz] = Wr[rows, qo:qo + qs].T
            out[:TD, 6, t, :sz] = Wc[rows, :].T
    return out.astype(BFNP)


def _prep_core(inputs, s):
    """Per-core input map (batch slice s)."""
    seq = np.asarray(inputs["sequence"], np.float32)[s]      # [B, L, 600]
    tr = np.asarray(inputs["transitions"])[s]
    th0 = np.asarray(inputs["th0"], np.float32)[s]
    tc0 = np.asarray(inputs["tc0"], np.float32)[s]

    tok_h = seq[:, :, :H]                                    # [B, L, 300]
    tok_c = seq[:, :, H:]
    is_left = (tr[:, 1::2].T == 2).astype(np.float32)[:K]    # [K, B]
    ml = is_left[None, :, None, :]                           # [1, K, 1, B]
    mr = 1.0 - ml

    tokh = np.zeros((128, L, 3, B), np.float32)
    tokc = np.zeros((128, L, 3, B), np.float32)
    for c, (off, sz) in enumerate(CH):
        tokh[:sz, :, c] = tok_h[:, :, off:off + sz].transpose(2, 1, 0)
        tokc[:sz, :, c] = tok_c[:, :, off:off + sz].transpose(2, 1, 0)
    hl = tokh[:, :K] * ml                                    # [128, K, 3, B]
    chl = tokc[:, :K] * ml
    chl[:, 0] = tokc[:, 0]          # k=0: right head (v_c) is token0 too
    mr3 = np.broadcast_to(mr, (128, K, 3, B))

    return dict(
        tokh=tokh.astype(BFNP), hl=hl.astype(BFNP),
        chl=np.ascontiguousarray(chl, np.float32),
        mr3=np.ascontiguousarray(mr3.astype(BFNP)),
        th0=np.ascontiguousarray(th0.T.astype(BFNP)),        # [64, B]
        tc0=np.ascontiguousarray(tc0.T),                     # [64, B] f32
    )


# --------------------------------------------------------------------------
# device program
# --------------------------------------------------------------------------
_CACHED_NC = None
CUR = ['init']            # label cell for analysis tooling


def _build_nc():
    nc = bacc.Bacc("TRN2", target_bir_lowering=False)
    tokh_d = nc.dram_tensor("tokh", [128, L, 3, B], bf16, kind="ExternalInput")
    hl_d = nc.dram_tensor("hl", [128, K, 3, B], bf16, kind="ExternalInput")
    chl_d = nc.dram_tensor("chl", [128, K, 3, B], f32, kind="ExternalInput")
    mr3_d = nc.dram_tensor("mr3", [128, K, 3, B], bf16, kind="ExternalInput")
    wtrk_d = nc.dram_tensor("wtrk", [128, 10, 2, 128], bf16,
                            kind="ExternalInput")
    wcmp_d = nc.dram_tensor("wcmp", [128, 7, NT, 128], bf16,
                            kind="ExternalInput")
    th0_d = nc.dram_tensor("th0", [TD, B], bf16, kind="ExternalInput")
    tc0_d = nc.dram_tensor("tc0", [TD, B], f32, kind="ExternalInput")
    outh_d = nc.dram_tensor("outh", [128, 3, B], f32, kind="ExternalOutput")

    with tile.TileContext(nc) as tc_:
        with (
            tc_.tile_pool(name="sg", bufs=1) as sg,
            tc_.tile_pool(name="st", bufs=3) as st,
            tc_.tile_pool(name="ps", bufs=1, space="PSUM") as ps,
        ):
            # ---------------- resident inputs ----------------
            tokh = sg.tile([128, L, 3, B], bf16)
            hl = sg.tile([128, K, 3, B], bf16)
            chl = sg.tile([128, K, 3, B], f32)
            mr3 = sg.tile([128, K, 3, B], bf16)
            wtrk = sg.tile([128, 10, 2, 128], bf16)
            wcmp = sg.tile([128, 7, NT, 128], bf16)
            th0 = sg.tile([TD, B], bf16)
            tc0 = sg.tile([128, B], f32)          # value at partitions 64:128

            nc.sync.dma_start(wtrk[:], wtrk_d[:])
            nc.sync.dma_start(wcmp[:], wcmp_d[:])
            nc.sync.dma_start(th0[:], th0_d[:])
            nc.sync.dma_start(tc0[TD:128, :], tc0_d[:])
            NG = 8
            for g in range(NG):
                lo, hi = g * (L // NG), (g + 1) * (L // NG)
                nc.sync.dma_start(tokh[:, lo:hi], tokh_d[:, lo:hi])
                klo, khi = min(lo, K), min(hi, K)
                if khi > klo:
                    nc.sync.dma_start(hl[:, klo:khi], hl_d[:, klo:khi])
                    nc.sync.dma_start(chl[:, klo:khi], chl_d[:, klo:khi])
                    nc.sync.dma_start(mr3[:, klo:khi], mr3_d[:, klo:khi])

            mm = nc.tensor.matmul

            # PSUM: one 2KB bank (per partition) per accumulator.
            # trk[ci][step]: [128, 2, 256] f32 (tiles t0/t1 at 1KB offsets
            # inside the bank); pcc[ci]: [128, 16, 32] f32 (gate tile t at
            # 128B offsets).
            trk = [[ps.tile([128, 2, 256], f32, name=f"trk{ci}{s}")
                    for s in range(2)] for ci in range(NCH)]
            pcc = [ps.tile([128, 16, 32], f32, name=f"pcc{ci}")
                   for ci in range(NCH)]

            chains = []
            for ci in range(NCH):
                sl = slice(ci * CB, (ci + 1) * CB)
                chains.append(dict(
                    ci=ci, sl=sl,
                    th=th0[:, sl],                  # bf16 [64, CB] moving
                    tc=tc0[:, sl],                  # f32, value at parts 64:
                    v=[tokh[0:sz, 0, c, sl] for c, (_, sz) in enumerate(CH)],
                    vc=None, vh3=None, hr=None, chd=None,
                ))

            # ---------------- emission helpers ----------------
            # Each (chain, step) tracking bank holds ONE accumulation window
            # per pair: start=True on the first token mm marks the whole 2KB
            # zero-region, every region's first touch writes fresh, and the
            # last-arriving contribution carries stop=True.  Emission order
            # must match data-arrival order (tok -> th -> v for step a,
            # tok -> v -> th for step b) so the stop mm dispatches last.
            def track_tok(ch, k, step):
                """Token mms; the first opens the bank window."""
                ci = ch["ci"]
                CUR[0] = f'ttok{ci}.{k}.{step}'
                ps_ = trk[ci][step]
                sl = ch["sl"]
                segs = [(0, k), (6, 0)] if step == 0 else [(0, k + 1), (3, k)]
                n = 0
                for t in range(2):
                    for q0, kk in segs:
                        for c, (_, sz) in enumerate(CH):
                            mm(ps_[:, t, 0:CB], wtrk[0:sz, q0 + c, t, :],
                               tokh[0:sz, kk, c, sl],
                               start=(n == 0), stop=False)
                            n += 1

            def track_th(ch, step, last):
                """W_hh @ th mms (accumulate; stop if final contribution)."""
                ci = ch["ci"]
                CUR[0] = f'tth{ci}.{step}'
                ps_ = trk[ci][step]
                for t in range(2):
                    mm(ps_[:, t, 0:CB], wtrk[0:TD, 9, t, :], ch["th"],
                       start=False, stop=(last and t == 1))

            def track_v(ch, step, last):
                """v mms (accumulate; stop if final contribution)."""
                ci = ch["ci"]
                CUR[0] = f'tv{ci}.{step}'
                ps_ = trk[ci][step]
                qv = 3 if step == 0 else 6
                for t in range(2):
                    for c, (_, sz) in enumerate(CH):
                        mm(ps_[:, t, 0:CB], wtrk[0:sz, qv + c, t, :],
                           ch["v"][c], start=False,
                           stop=(last and t == 1 and c == 2))

            def track_step(ch, step):
                """Sigmoid + LSTM cell tail. sa: [:,0]=[i|f], [:,1]=[2g|o].
                tc state lives at partitions 64:128; th output at 0:64."""
                ci = ch["ci"]
                CUR[0] = f'ttail{step}{ci}'
                ps_ = trk[ci][step]
                sa = st.tile([128, 2, CB], bf16, tag=f"sa{step}{ci}")
                nc.scalar.activation(sa[:], ps_[:, :, 0:CB], AF.Sigmoid)
                si = sa[0:TD, 0, :]
                sf = sa[TD:128, 0, :]
                sg2 = sa[0:TD, 1, :]
                so = sa[TD:128, 1, :]
                m = st.tile([TD, CB], bf16, tag=f"m{ci}")
                nc.vector.tensor_tensor(m[:], si, sg2, alu.mult)
                q = st.tile([128, CB], bf16, tag=f"q{ci}")
                nc.vector.scalar_tensor_tensor(q[TD:128, :], m[:], 2.0, si,
                                               alu.mult, alu.subtract)
                r = st.tile([128, CB], f32, tag=f"r{ci}")
                nc.gpsimd.tensor_tensor(r[TD:128, :], sf, ch["tc"][TD:128, :],
                                        alu.mult)
                tcn = st.tile([128, CB], f32, tag=f"tc{ci}")
                nc.vector.tensor_tensor(tcn[TD:128, :], q[TD:128, :],
                                        r[TD:128, :], alu.add)
                ttc = st.tile([128, CB], bf16, tag=f"stc{ci}")
                nc.scalar.activation(ttc[TD:128, :], tcn[TD:128, :], AF.Tanh)
                thn = st.tile([TD, CB], bf16, tag=f"th{step}{ci}")
                nc.vector.tensor_tensor(thn[:], so, ttc[TD:128, :], alu.mult)
                ch["tc"] = tcn
                ch["th"] = thn[:]

            # The comp bank likewise holds ONE window per pair: hl mms open
            # it (available immediately), hr mms accumulate once v(k-1)
            # lands, and the 15 W_c@th_b mms close it after th_b — so only
            # those 15 mms sit on the critical path.
            def comp_fill_hl(ch, k):
                ci = ch["ci"]
                CUR[0] = f'chl{ci}.{k}'
                sl = ch["sl"]
                pc = pcc[ci]
                for t in range(NT):
                    for c, (_, sz) in enumerate(CH):
                        mm(pc[:, t, 0:CB], wcmp[0:sz, c, t, :],
                           hl[0:sz, k, c, sl], start=(t == 0 and c == 0),
                           stop=False)

            def comp_fill_hr(ch, k):
                ci = ch["ci"]
                CUR[0] = f'chr{ci}.{k}'
                pc = pcc[ci]
                hr = ch["hr"]
                for t in range(NT):
                    for c, (_, sz) in enumerate(CH):
                        mm(pc[:, t, 0:CB], wcmp[0:sz, 3 + c, t, :],
                           hr[0:sz, c, :], start=False, stop=False)

            def comp_close(ch):
                ci = ch["ci"]
                CUR[0] = f'cwc{ci}'
                pc = pcc[ci]
                for t in range(NT):
                    mm(pc[:, t, 0:CB], wcmp[0:TD, 6, t, :], ch["th"],
                       start=False, stop=(t == NT - 1))

            def comp_op_hr(ch, k):
                """hr = mr3[k] * v   (bf16, one op over all chunks)."""
                ci = ch["ci"]
                CUR[0] = f'hrop{ci}.{k}'
                sl = ch["sl"]
                hr = st.tile([128, 3, CB], bf16, tag=f"hr{ci}")
                vsrc = tokh[:, 0, :, sl] if k == 0 else ch["vh3"][:]
                nc.vector.tensor_tensor(hr[:], vsrc, mr3[:, k, :, sl],
                                        alu.mult)
                ch["hr"] = hr

            def comp_op_ch(ch, k):
                """c_head = chl[k] + mr3[k]*vc   (f32, [128,3,CB])."""
                ci = ch["ci"]
                CUR[0] = f'chop{ci}.{k}'
                sl = ch["sl"]
                if k == 0:
                    ch["chd"] = chl[:, 0, :, sl]
                    return
                w = st.tile([128, 3, CB], f32, tag=f"w{ci}")
                nc.gpsimd.tensor_tensor(w[:], ch["vc"][:], mr3[:, k, :, sl],
                                        alu.mult)
                chd = st.tile([128, 3, CB], f32, tag=f"chd{ci}")
                nc.vector.tensor_tensor(chd[:], w[:], chl[:, k, :, sl],
                                        alu.add)
                ch["chd"] = chd[:]

            def comp_tail(ch, k):
                """Composition cell tail; produces vh3 (bf16) + vc (f32).
                tiles: 0:3 fh, 3:6 fc, 6:9 i, 9:12 2u, 12:15 o."""
                ci = ch["ci"]
                CUR[0] = f'ctail{ci}.{k}'
                pc = pcc[ci]
                sc = st.tile([128, NT, CB], bf16, tag=f"sc{ci}")
                nc.scalar.activation(sc[:], pc[:, 0:NT, 0:CB], AF.Sigmoid)
                A = st.tile([128, 3, CB], bf16, tag=f"A{ci}")
                nc.gpsimd.tensor_tensor(A[:], sc[:, 0:3, :], sc[:, 3:6, :],
                                        alu.add)
                t3 = st.tile([128, 3, CB], f32, tag=f"t3{ci}")
                nc.gpsimd.tensor_tensor(t3[:], A[:], ch["chd"], alu.mult)
                m2 = st.tile([128, 3, CB], bf16, tag=f"m2{ci}")
                nc.vector.tensor_tensor(m2[:], sc[:, 6:9, :], sc[:, 9:12, :],
                                        alu.mult)
                q2 = st.tile([128, 3, CB], bf16, tag=f"q2{ci}")
                nc.vector.scalar_tensor_tensor(q2[:], m2[:], 2.0,
                                               sc[:, 6:9, :],
                                               alu.mult, alu.subtract)
                cj = st.tile([128, 3, CB], f32, tag=f"cj{ci}")
                nc.vector.tensor_tensor(cj[:], q2[:], t3[:], alu.add)
                tcc = st.tile([128, 3, CB], bf16, tag=f"scc{ci}")
                nc.scalar.activation(tcc[:], cj[:], AF.Tanh)
                vh3 = st.tile([128, 3, CB], bf16, tag=f"vh3{ci}")
                nc.vector.tensor_tensor(vh3[:], sc[:, 12:15, :], tcc[:],
                                        alu.mult)
                ch["vh3"] = vh3
                ch["vc"] = cj
                ch["v"] = [vh3[0:sz, c, :] for c, (_, sz) in enumerate(CH)]
                if k == K - 1:
                    vhf = st.tile([128, 3, CB], f32, tag=f"vhf{ci}")
                    nc.vector.tensor_tensor(vhf[:], sc[:, 12:15, :], tcc[:],
                                            alu.mult)
                    nc.sync.dma_start(outh_d[:, :, ch["sl"]], vhf[:])

            # ---------------- main schedule ----------------
            # Per pair k, per chain: windows are emitted in data-arrival
            # order so each PSUM bank's group windows serialize without
            # blocking: tok -> th -> v (tracking), hl -> hr -> wc (comp).
            for ch in chains:
                comp_op_hr(ch, 0)
                comp_op_ch(ch, 0)
            for k in range(K):
                for ch in chains:
                    # step a: tok already ordered first; th available from
                    # pair k-1 step b; v(k-1) closes the group.
                    track_tok(ch, k, 0)
                    comp_fill_hl(ch, k)
                    track_th(ch, 0, last=False)
                    track_v(ch, 0, last=True)
                    comp_fill_hr(ch, k)
                    track_step(ch, 0)               # sigma-a -> th_a
                    # step b: v(k-1) before th_a.
                    track_tok(ch, k, 1)
                    track_v(ch, 1, last=False)
                    track_th(ch, 1, last=True)
                    track_step(ch, 1)               # sigma-b -> th_b
                    comp_close(ch)                  # W_c @ th_b
                    comp_tail(ch, k)                # sigma-c -> v(k)
                    if k + 1 < K:
                        comp_op_hr(ch, k + 1)
                        comp_op_ch(ch, k + 1)

    nc.compile()
    return nc


def _get_nc():
    global _CACHED_NC
    if _CACHED_NC is None:
        _CACHED_NC = _build_nc()
    return _CACHED_NC


# --------------------------------------------------------------------------
# host wrapper
# --------------------------------------------------------------------------
def make_in_maps(inputs):
    wtrk = _track_w(np.asarray(inputs["W_ih"], np.float32),
                    np.asarray(inputs["W_hh"], np.float32))
    wcmp = _comp_w(np.asarray(inputs["W_c"], np.float32),
                   np.asarray(inputs["Uh_w"], np.float32),
                   np.asarray(inputs["Ul_w"], np.float32),
                   np.asarray(inputs["Ur_w"], np.float32))
    in_maps = []
    for i in range(NCORES):
        s = slice(i * B, (i + 1) * B)
        d = _prep_core(inputs, s)
        d.update(wtrk=wtrk, wcmp=wcmp)
        in_maps.append(d)
    return in_maps


def assemble_out(res_list):
    outs = []
    for r in res_list:
        arr = r["outh"]                      # [128, 3, B]
        o = np.empty((B, H), np.float32)
        for c, (off, sz) in enumerate(CH):
            o[:, off:off + sz] = arr[0:sz, c, :].T
        outs.append(o)
    return np.concatenate(outs, axis=0)


def kernel(**inputs) -> np.ndarray:
    nc = _get_nc()
    in_maps = make_in_maps(inputs)
    res = run_bass_kernel_spmd(nc, in_maps, core_ids=list(range(NCORES)))
    return assemble_out(res.results)
